# revision 31
# baseline (speedup 1.0000x reference)
"""LSTM (B=131072, T=10, INP=HID=64) + linear head, data-parallel on 8 TRN2 cores.

v4: dual-half fp8e4m3 DoubleRow matmuls with hi/lo error compensation +
software-pipelined emission. Per core (B_loc=16384), batch is split into 16
units x 2 groups (A/B) of 512 columns.

  - Feature-major on chip: each PSUM gate bank [128, 512] holds
    [gate_A(0:64); gate_B(64:128)] so elementwise ops run 128 lanes.
  - DoubleRow matmuls must write PSUM starting at partition 0, so each DR
    computes BOTH halves at once with zero-padded weights: rhs slot0 carries
    the A-group vector (A-weights occupy output cols 0:64, cols 64:128 zero),
    slot1 the B-group (cols 64:128). 3 DR matmuls per gate bank:
      hi:   W_hi @ v_hi            (v = [h_q8; x_hi] per half)
      lo:   W_hi @ x_lo + bias     (bias via two ones-rows: b_hi, b_lo)
      comp: W_lo @ v_hi
    at out-free 512 x 0.5 cycles/row = ~107ns each -> 12 matmuls/unit-step.
  - rhs tiles [128, 2(kind hi/lo), 2(half), 512] fp8: x regions stream from
    DRAM (one DMA per half covering both kinds), h_q8 written by GPSIMD casts.
    Whole tiles are initialized by one DMA each from a host image (parity 0
    with x(t=0) and h=0; parity 1 with x(t=1)), so no memsets at all.
  - ACT: one merged 4-bank sigmoid per unit (g weights pre-doubled so
    tanh(g)=2*sig(2g)-1) and one tanh per unit QUAD ([128,4,512] c tiles);
    tanh is emitted one quad behind sigmoid so the in-order ACT queue never
    stalls on the DVE gate-algebra chain.
  - DVE: Gt (tensor_scalar, 4x mode), i*g, f*c, c-add, h=o*tanh (2x mode).
  - GPSIMD (Pool): bf16->fp8 h casts into the next step's rhs tiles.
  - The output head (t=9) is a drain phase after the main loop so its PSUM
    tiles never interleave with the gate-bank pool rotation.
  - Numerics (numpy model of this exact scheme): rel err ~7.7e-3 vs fp32.
"""

import numpy as np
import ml_dtypes

import concourse.bass as bass
import concourse.mybir as mybir
from concourse import bacc
import concourse.tile as tile

HID = 64
INP = 64
T = 10
B = 131072
NCORES = 8
B_LOC = B // NCORES  # 16384
NB = 512             # batch columns per group
NU = B_LOC // (2 * NB)  # 16 units (A+B groups each)
NQ = NU // 4            # 4 tanh quads

BF = mybir.dt.bfloat16
F8 = mybir.dt.float8e4
F32 = mybir.dt.float32
AF = mybir.ActivationFunctionType
ALU = mybir.AluOpType
DR = mybir.MatmulPerfMode.DoubleRow

# psum gate-bank order: 0=i, 1=f, 2=o, 3=g ; torch block order i,f,g,o
SLICE_TO_TORCH_GATE = [0, 1, 3, 2]


def emit_lstm(tc, aps):
    nc = tc.nc
    xt, xinit, Wd, WOd, BOd, y = (
        aps["xt"], aps["xinit"], aps["Wd"], aps["WOd"], aps["BOd"], aps["y"])

    with (
        tc.tile_pool(name="const", bufs=1) as cpool,
        tc.tile_pool(name="work", bufs=10) as gpool,
        tc.tile_pool(name="small", bufs=3) as wpool,
        tc.tile_pool(name="psum", bufs=2, space="PSUM") as ppool,
    ):
        W_sb = cpool.tile([128, 2, 4, 3, 128], F8, tag="W")
        WO_sb = cpool.tile([128, 2], BF, tag="WO")
        BO_sb = cpool.tile([2, 1], F32, tag="BO")

        # rhs tiles per (unit, parity): [k, kind(hi/lo), half(A/B), col]
        R = [[cpool.tile([128, 2, 2, NB], F8, tag=f"R{u}_{p}", name=f"R{u}_{p}")
              for p in range(2)] for u in range(NU)]
        C = [cpool.tile([128, 4, NB], BF, tag=f"C{k}", name=f"C{k}")
             for k in range(NQ)]
        H9 = [cpool.tile([128, 4, NB], BF, tag=f"H9_{k}", name=f"H9_{k}")
              for k in range(NQ)]

        gs_ref = [None] * NU  # per-unit sigmoid output tile of the current step

        def flush_quad(t, k):
            """tanh + h for quad k (units 4k..4k+3) of step t (lagged)."""
            th = wpool.tile([128, 4, NB], BF, tag="th", name=f"th_{t}_{k}")
            nc.scalar.activation(th, C[k], AF.Tanh)
            for j in range(4):
                u = 4 * k + j
                if t < T - 1:
                    H = wpool.tile([128, NB], BF, tag="H", name=f"h_{t}_{u}")
                    nc.vector.tensor_mul(H, gs_ref[u][:, 2], th[:, j])
                    nxt = (t + 1) % 2
                    nc.gpsimd.tensor_copy(out=R[u][nxt][0:64, 0, 0, :],
                                          in_=H[0:64, :])
                    nc.gpsimd.tensor_copy(out=R[u][nxt][64:128, 0, 1, :],
                                          in_=H[64:128, :])
                else:
                    nc.vector.tensor_mul(H9[k][:, j], gs_ref[u][:, 2],
                                         th[:, j])

        def head_quad(k):
            """y = W_out @ h + b_out for quad k, one DMA per quad."""
            op = ppool.tile([2, 4, NB], F32, tag="g", name=f"op_{k}")
            for j in range(4):
                nc.tensor.matmul(op[:, j], WO_sb, H9[k][:, j], start=True,
                                 stop=True, skip_group_check=True)
            ob = wpool.tile([2, 4, NB], F32, tag="ob", name=f"ob_{k}")
            if k % 2 == 0:
                nc.scalar.activation(ob, op, AF.Identity, bias=BO_sb)
            else:
                nc.vector.tensor_scalar(ob, op, BO_sb, None, ALU.add)
            nc.sync.dma_start(
                out=y[8 * k * NB:8 * (k + 1) * NB].rearrange(
                    "(q p n) -> p q n", q=4, p=2),
                in_=ob)

        # whole-tile init: parity 0 carries x(t=0) and h=0, parity 1 carries
        # x(t=1) (h regions are placeholder zeros, overwritten by the t=0
        # casts before they are read at t=1)
        nc.sync.dma_start(out=W_sb, in_=Wd)
        for p in range(2):
            for u in range(NU):
                nc.sync.dma_start(
                    out=R[u][p],
                    in_=xinit[p, :, :, :, u * NB:(u + 1) * NB])
        nc.sync.dma_start(out=WO_sb, in_=WOd)
        nc.sync.dma_start(out=BO_sb, in_=BOd)

        for t in range(T):
            par, nxt = t % 2, (t + 1) % 2
            for u in range(NU):
                k, j = divmod(u, 4)
                if 0 < t < T - 1:
                    # x for step t+1 (>= 2); both kinds in one DMA per half
                    nc.sync.dma_start(
                        out=R[u][nxt][64:128, :, 0, :],
                        in_=xt[t + 1, :, :, 2 * u * NB:(2 * u + 1) * NB])
                    nc.sync.dma_start(
                        out=R[u][nxt][0:64, :, 1, :],
                        in_=xt[t + 1, :, :, (2 * u + 1) * NB:(2 * u + 2) * NB])

                ps = ppool.tile([128, 4, NB], F32, tag="g", name=f"ps_{t}_{u}")
                hi = R[u][par][:, 0, :, :]
                lo = R[u][par][:, 1, :, :]
                for s in range(4):
                    nc.tensor.matmul(ps[:, s], W_sb[:, :, s, 0, :], hi,
                                     start=True, stop=False, perf_mode=DR)
                    nc.tensor.matmul(ps[:, s], W_sb[:, :, s, 1, :], lo,
                                     start=False, stop=False, perf_mode=DR)
                    nc.tensor.matmul(ps[:, s], W_sb[:, :, s, 2, :], hi,
                                     start=False, stop=True, perf_mode=DR)

                GS = gpool.tile([128, 4, NB], BF, tag="GS", name=f"gs_{t}_{u}")
                nc.scalar.activation(GS, ps, AF.Sigmoid)
                gs_ref[u] = GS

                Gt = wpool.tile([128, NB], BF, tag="Gt", name=f"gt_{t}_{u}")
                nc.vector.tensor_scalar(Gt, GS[:, 3], 2.0, -1.0, ALU.mult,
                                        ALU.add)
                if t == 0:
                    nc.vector.tensor_mul(C[k][:, j], GS[:, 0], Gt)
                else:
                    uu = wpool.tile([128, NB], BF, tag="uu", name=f"uu_{t}_{u}")
                    ww = wpool.tile([128, NB], BF, tag="ww", name=f"ww_{t}_{u}")
                    nc.vector.tensor_mul(uu, GS[:, 0], Gt)
                    nc.vector.tensor_mul(ww, GS[:, 1], C[k][:, j])
                    nc.vector.tensor_add(C[k][:, j], uu, ww)

                if u % 4 == 3:
                    if u == 3:
                        if t > 0:
                            flush_quad(t - 1, NQ - 1)
                    else:
                        flush_quad(t, (u - 7) // 4)
                if t == T - 1 and u == NU - 1:
                    head_quad(0)
                    head_quad(1)
        flush_quad(T - 1, NQ - 1)
        head_quad(2)
        head_quad(3)


def _q8(a):
    """e4m3 hi/lo split of a float32 array -> (hi, lo) float32."""
    e4 = ml_dtypes.float8_e4m3
    hi = a.astype(e4).astype(np.float32)
    lo = (a - hi).astype(e4).astype(np.float32)
    return hi, lo


def prep_weights(W_ih, W_hh, b_ih, b_hh, W_out, b_out):
    """Host-side packing into dual-half zero-padded DR stacks.

    Wd dims: (k, slot=half, s, kind, m). kind 0 = W_hi, 1 = W_hi@x_lo + bias
    ones-rows, 2 = W_lo. A-half weights occupy m-cols 0:64, B-half 64:128.
    """
    e4 = ml_dtypes.float8_e4m3
    bf16 = ml_dtypes.bfloat16
    Wd = np.zeros((128, 2, 4, 3, 128), np.float32)
    b_all = (b_ih + b_hh).astype(np.float32)
    for s, gi in enumerate(SLICE_TO_TORCH_GATE):
        scale = 2.0 if s == 3 else 1.0
        Wih = W_ih[gi * 64:(gi + 1) * 64, :].astype(np.float32) * scale
        Whh = W_hh[gi * 64:(gi + 1) * 64, :].astype(np.float32) * scale
        b = b_all[gi * 64:(gi + 1) * 64] * scale
        Wih_hi, Wih_lo = _q8(Wih)
        Whh_hi, Whh_lo = _q8(Whh)
        b_hi, b_lo = _q8(b)
        # A half: rhs rows [h; x], out cols 0:64
        Wd[0:64, 0, s, 0, 0:64] = Whh_hi.T
        Wd[64:128, 0, s, 0, 0:64] = Wih_hi.T
        Wd[0, 0, s, 1, 0:64] = b_hi
        Wd[1, 0, s, 1, 0:64] = b_lo
        Wd[64:128, 0, s, 1, 0:64] = Wih_hi.T
        Wd[0:64, 0, s, 2, 0:64] = Whh_lo.T
        Wd[64:128, 0, s, 2, 0:64] = Wih_lo.T
        # B half: rhs rows [x; h], out cols 64:128
        Wd[0:64, 1, s, 0, 64:128] = Wih_hi.T
        Wd[64:128, 1, s, 0, 64:128] = Whh_hi.T
        Wd[0:64, 1, s, 1, 64:128] = Wih_hi.T
        Wd[64, 1, s, 1, 64:128] = b_hi
        Wd[65, 1, s, 1, 64:128] = b_lo
        Wd[0:64, 1, s, 2, 64:128] = Wih_lo.T
        Wd[64:128, 1, s, 2, 64:128] = Whh_lo.T
    WO = np.zeros((128, 2), np.float32)
    WO[0:64, 0] = W_out[0].astype(np.float32)
    WO[64:128, 1] = W_out[0].astype(np.float32)
    BO = np.full((2, 1), np.float32(b_out[0]))
    return {
        "Wd": Wd.astype(e4),
        "WOd": WO.astype(bf16),
        "BOd": BO,
    }


_BUILD_CACHE = {}


def build_nc(merged=True):
    key = ("nc", merged)
    if key in _BUILD_CACHE:
        return _BUILD_CACHE[key]
    nc = bacc.Bacc("TRN2", target_bir_lowering=False, debug=False)
    aps = {
        "xt": nc.dram_tensor("xt", [T, INP, 2, B_LOC], F8,
                             kind="ExternalInput").ap(),
        "xinit": nc.dram_tensor("xinit", [2, 128, 2, 2, B_LOC // 2], F8,
                                kind="ExternalInput").ap(),
        "Wd": nc.dram_tensor("Wd", [128, 2, 4, 3, 128], F8,
                             kind="ExternalInput").ap(),
        "WOd": nc.dram_tensor("WOd", [128, 2], BF, kind="ExternalInput").ap(),
        "BOd": nc.dram_tensor("BOd", [2, 1], F32, kind="ExternalInput").ap(),
        "y": nc.dram_tensor("y", [B_LOC], F32, kind="ExternalOutput").ap(),
    }
    with tile.TileContext(nc) as tc:
        emit_lstm(tc, aps)
    nc.compile()
    _BUILD_CACHE[key] = nc
    return nc


def make_in_maps(x, W_ih, W_hh, b_ih, b_hh, W_out, b_out, merged=True):
    e4 = ml_dtypes.float8_e4m3
    wd = prep_weights(W_ih, W_hh, b_ih, b_hh, W_out, b_out)
    # [B, T, I] -> [T, I, B]; hi/lo e4m3 split stacked on a new axis
    xT = np.ascontiguousarray(x.transpose(1, 2, 0)).astype(np.float32)
    hi = xT.astype(e4).astype(np.float32)
    lo = (xT - hi).astype(e4)
    xt = np.stack([hi.astype(e4), lo], axis=2)  # [T, I, 2, B]

    # whole-tile init images [par, k, kind, half, B/2]: x(t=par) + h=0 +
    # lo-slot constants (ones rows for the bias, zeros elsewhere)
    B_half = B // 2
    xinit = np.zeros((2, 128, 2, 2, B_half), e4)
    xg = xt.reshape(T, INP, 2, B // (2 * NB), 2, NB)  # [...groups, half, col]
    for p in range(2):
        xA = xg[p, :, :, :, 0, :].reshape(INP, 2, B_half)
        xB = xg[p, :, :, :, 1, :].reshape(INP, 2, B_half)
        xinit[p, 64:128, :, 0, :] = xA  # A half: x at rows 64:128, both kinds
        xinit[p, 0:64, :, 1, :] = xB    # B half: x at rows 0:64
        xinit[p, 0, 1, 0, :] = e4(1.0)   # A lo ones-rows (bias)
        xinit[p, 1, 1, 0, :] = e4(1.0)
        xinit[p, 64, 1, 1, :] = e4(1.0)  # B lo ones-rows
        xinit[p, 65, 1, 1, :] = e4(1.0)

    in_maps = []
    for c in range(NCORES):
        sl = np.ascontiguousarray(xt[:, :, :, c * B_LOC:(c + 1) * B_LOC])
        sli = np.ascontiguousarray(
            xinit[:, :, :, :, c * (B_LOC // 2):(c + 1) * (B_LOC // 2)])
        in_maps.append({"xt": sl, "xinit": sli, **wd})
    return in_maps


def kernel(x, W_ih, W_hh, b_ih, b_hh, W_out, b_out):
    from concourse.bass_utils import run_bass_kernel_spmd

    nc = build_nc()
    in_maps = make_in_maps(x, W_ih, W_hh, b_ih, b_hh, W_out, b_out)
    res = run_bass_kernel_spmd(nc, in_maps, core_ids=list(range(NCORES)))
    y = np.concatenate([res.results[c]["y"] for c in range(NCORES)])
    return y.reshape(B, 1).astype(np.float32)


# revision 34
# speedup vs baseline: 1.0198x; 1.0198x over previous
"""LSTM (B=131072, T=10, INP=HID=64) + linear head, data-parallel on 8 TRN2 cores.

v4: dual-half fp8e4m3 DoubleRow matmuls with hi/lo error compensation +
software-pipelined emission. Per core (B_loc=16384), batch is split into 16
units x 2 groups (A/B) of 512 columns.

  - Feature-major on chip: each PSUM gate bank [128, 512] holds
    [gate_A(0:64); gate_B(64:128)] so elementwise ops run 128 lanes.
  - DoubleRow matmuls must write PSUM starting at partition 0, so each DR
    computes BOTH halves at once with zero-padded weights: rhs slot0 carries
    the A-group vector (A-weights occupy output cols 0:64, cols 64:128 zero),
    slot1 the B-group (cols 64:128). 3 DR matmuls per gate bank:
      hi:   W_hi @ v_hi            (v = [h_q8; x_hi] per half)
      lo:   W_hi @ x_lo + bias     (bias via two ones-rows: b_hi, b_lo)
      comp: W_lo @ v_hi
    at out-free 512 x 0.5 cycles/row = ~107ns each -> 12 matmuls/unit-step.
  - rhs tiles [128, 2(kind hi/lo), 2(half), 512] fp8: x regions stream from
    DRAM (one DMA per half covering both kinds), h_q8 written by GPSIMD casts.
    Whole tiles are initialized by one DMA each from a host image (parity 0
    with x(t=0) and h=0; parity 1 with x(t=1)), so no memsets at all.
  - ACT: one merged 4-bank sigmoid per unit (g weights pre-doubled so
    tanh(g)=2*sig(2g)-1) and one tanh per unit QUAD ([128,4,512] c tiles);
    tanh is emitted one quad behind sigmoid so the in-order ACT queue never
    stalls on the DVE gate-algebra chain.
  - DVE: Gt (tensor_scalar, 4x mode), i*g, f*c, c-add, h=o*tanh (2x mode).
  - GPSIMD (Pool): bf16->fp8 h casts into the next step's rhs tiles.
  - The output head (t=9) is a drain phase after the main loop so its PSUM
    tiles never interleave with the gate-bank pool rotation.
  - Numerics (numpy model of this exact scheme): rel err ~7.7e-3 vs fp32.
"""

import numpy as np
import ml_dtypes

import concourse.bass as bass
import concourse.mybir as mybir
from concourse import bacc
import concourse.tile as tile

HID = 64
INP = 64
T = 10
B = 131072
NCORES = 8
B_LOC = B // NCORES  # 16384
NB = 512             # batch columns per group
NU = B_LOC // (2 * NB)  # 16 units (A+B groups each)
NQ = NU // 4            # 4 tanh quads

BF = mybir.dt.bfloat16
F8 = mybir.dt.float8e4
F32 = mybir.dt.float32
AF = mybir.ActivationFunctionType
ALU = mybir.AluOpType
DR = mybir.MatmulPerfMode.DoubleRow

# psum gate-bank order: 0=f, 1=i, 2=o, 3=g ; torch block order i,f,g,o.
# f sits at bank 0 so the t=0 sigmoid (which never needs f: c_prev=0) can
# cover just banks 1:4.
SLICE_TO_TORCH_GATE = [1, 0, 3, 2]


def emit_lstm(tc, aps):
    nc = tc.nc
    xt, xinit, Wd, WOd, BOd, y = (
        aps["xt"], aps["xinit"], aps["Wd"], aps["WOd"], aps["BOd"], aps["y"])

    with (
        tc.tile_pool(name="const", bufs=1) as cpool,
        tc.tile_pool(name="work", bufs=10) as gpool,
        tc.tile_pool(name="small", bufs=3) as wpool,
        tc.tile_pool(name="psum", bufs=2, space="PSUM") as ppool,
    ):
        W_sb = cpool.tile([128, 2, 4, 3, 128], F8, tag="W")
        WO_sb = cpool.tile([128, 2], BF, tag="WO")
        BO_sb = cpool.tile([2, 1], F32, tag="BO")

        # rhs tiles per (unit, parity): [k, kind(hi/lo), half(A/B), col]
        R = [[cpool.tile([128, 2, 2, NB], F8, tag=f"R{u}_{p}", name=f"R{u}_{p}")
              for p in range(2)] for u in range(NU)]
        C = [cpool.tile([128, 4, NB], BF, tag=f"C{k}", name=f"C{k}")
             for k in range(NQ)]
        H9 = [cpool.tile([128, 4, NB], BF, tag=f"H9_{k}", name=f"H9_{k}")
              for k in range(NQ)]

        gs_ref = [None] * NU  # per-unit sigmoid output tile of the current step

        def flush_quad(t, k):
            """tanh + h for quad k (units 4k..4k+3) of step t (lagged)."""
            th = wpool.tile([128, 4, NB], BF, tag="th", name=f"th_{t}_{k}")
            nc.scalar.activation(th, C[k], AF.Tanh)
            for j in range(4):
                u = 4 * k + j
                if t < T - 1:
                    H = wpool.tile([128, NB], BF, tag="H", name=f"h_{t}_{u}")
                    nc.vector.tensor_mul(H, gs_ref[u][:, 2], th[:, j])
                    nxt = (t + 1) % 2
                    nc.gpsimd.tensor_copy(out=R[u][nxt][0:64, 0, 0, :],
                                          in_=H[0:64, :])
                    nc.gpsimd.tensor_copy(out=R[u][nxt][64:128, 0, 1, :],
                                          in_=H[64:128, :])
                else:
                    nc.vector.tensor_mul(H9[k][:, j], gs_ref[u][:, 2],
                                         th[:, j])

        def head_quad(k):
            """y = W_out @ h + b_out for quad k, one DMA per quad."""
            op = ppool.tile([2, 4, NB], F32, tag="g", name=f"op_{k}")
            for j in range(4):
                nc.tensor.matmul(op[:, j], WO_sb, H9[k][:, j], start=True,
                                 stop=True, skip_group_check=True)
            ob = wpool.tile([2, 4, NB], F32, tag="ob", name=f"ob_{k}")
            if k % 2 == 0:
                nc.scalar.activation(ob, op, AF.Identity, bias=BO_sb)
            else:
                nc.vector.tensor_scalar(ob, op, BO_sb, None, ALU.add)
            nc.sync.dma_start(
                out=y[8 * k * NB:8 * (k + 1) * NB].rearrange(
                    "(q p n) -> p q n", q=4, p=2),
                in_=ob)

        # whole-tile init: parity 0 carries x(t=0) and h=0, parity 1 carries
        # x(t=1) (h regions are placeholder zeros, overwritten by the t=0
        # casts before they are read at t=1)
        nc.sync.dma_start(out=W_sb, in_=Wd)
        for p in range(2):
            for u in range(NU):
                nc.sync.dma_start(
                    out=R[u][p],
                    in_=xinit[p, :, :, :, u * NB:(u + 1) * NB])
        nc.sync.dma_start(out=WO_sb, in_=WOd)
        nc.sync.dma_start(out=BO_sb, in_=BOd)

        for t in range(T):
            par, nxt = t % 2, (t + 1) % 2
            for u in range(NU):
                k, j = divmod(u, 4)
                if 0 < t < T - 1:
                    # x for step t+1 (>= 2); both kinds in one DMA per half
                    nc.sync.dma_start(
                        out=R[u][nxt][64:128, :, 0, :],
                        in_=xt[t + 1, :, :, 2 * u * NB:(2 * u + 1) * NB])
                    nc.sync.dma_start(
                        out=R[u][nxt][0:64, :, 1, :],
                        in_=xt[t + 1, :, :, (2 * u + 1) * NB:(2 * u + 2) * NB])

                ps = ppool.tile([128, 4, NB], F32, tag="g", name=f"ps_{t}_{u}")
                hi = R[u][par][:, 0, :, :]
                lo = R[u][par][:, 1, :, :]
                for s in (range(1, 4) if t == 0 else range(4)):
                    nc.tensor.matmul(ps[:, s], W_sb[:, :, s, 0, :], hi,
                                     start=True, stop=False, perf_mode=DR)
                    nc.tensor.matmul(ps[:, s], W_sb[:, :, s, 1, :], lo,
                                     start=False, stop=False, perf_mode=DR)
                    nc.tensor.matmul(ps[:, s], W_sb[:, :, s, 2, :], hi,
                                     start=False, stop=True, perf_mode=DR)

                GS = gpool.tile([128, 4, NB], BF, tag="GS", name=f"gs_{t}_{u}")
                if t == 0:
                    nc.scalar.activation(GS[:, 1:4], ps[:, 1:4], AF.Sigmoid)
                else:
                    nc.scalar.activation(GS, ps, AF.Sigmoid)
                gs_ref[u] = GS

                Gt = wpool.tile([128, NB], BF, tag="Gt", name=f"gt_{t}_{u}")
                nc.vector.tensor_scalar(Gt, GS[:, 3], 2.0, -1.0, ALU.mult,
                                        ALU.add)
                if t == 0:
                    nc.vector.tensor_mul(C[k][:, j], GS[:, 1], Gt)
                else:
                    uu = wpool.tile([128, NB], BF, tag="uu", name=f"uu_{t}_{u}")
                    ww = wpool.tile([128, NB], BF, tag="ww", name=f"ww_{t}_{u}")
                    nc.vector.tensor_mul(uu, GS[:, 1], Gt)
                    nc.vector.tensor_mul(ww, GS[:, 0], C[k][:, j])
                    nc.vector.tensor_add(C[k][:, j], uu, ww)

                if u % 4 == 3:
                    if u == 3:
                        if t > 0:
                            flush_quad(t - 1, NQ - 1)
                    else:
                        flush_quad(t, (u - 7) // 4)
                if t == T - 1 and u == NU - 1:
                    head_quad(0)
                    head_quad(1)
        flush_quad(T - 1, NQ - 1)
        head_quad(2)
        head_quad(3)


def _q8(a):
    """e4m3 hi/lo split of a float32 array -> (hi, lo) float32."""
    e4 = ml_dtypes.float8_e4m3
    hi = a.astype(e4).astype(np.float32)
    lo = (a - hi).astype(e4).astype(np.float32)
    return hi, lo


def prep_weights(W_ih, W_hh, b_ih, b_hh, W_out, b_out):
    """Host-side packing into dual-half zero-padded DR stacks.

    Wd dims: (k, slot=half, s, kind, m). kind 0 = W_hi, 1 = W_hi@x_lo + bias
    ones-rows, 2 = W_lo. A-half weights occupy m-cols 0:64, B-half 64:128.
    """
    e4 = ml_dtypes.float8_e4m3
    bf16 = ml_dtypes.bfloat16
    Wd = np.zeros((128, 2, 4, 3, 128), np.float32)
    b_all = (b_ih + b_hh).astype(np.float32)
    for s, gi in enumerate(SLICE_TO_TORCH_GATE):
        scale = 2.0 if s == 3 else 1.0
        Wih = W_ih[gi * 64:(gi + 1) * 64, :].astype(np.float32) * scale
        Whh = W_hh[gi * 64:(gi + 1) * 64, :].astype(np.float32) * scale
        b = b_all[gi * 64:(gi + 1) * 64] * scale
        Wih_hi, Wih_lo = _q8(Wih)
        Whh_hi, Whh_lo = _q8(Whh)
        b_hi, b_lo = _q8(b)
        # A half: rhs rows [h; x], out cols 0:64
        Wd[0:64, 0, s, 0, 0:64] = Whh_hi.T
        Wd[64:128, 0, s, 0, 0:64] = Wih_hi.T
        Wd[0, 0, s, 1, 0:64] = b_hi
        Wd[1, 0, s, 1, 0:64] = b_lo
        Wd[64:128, 0, s, 1, 0:64] = Wih_hi.T
        Wd[0:64, 0, s, 2, 0:64] = Whh_lo.T
        Wd[64:128, 0, s, 2, 0:64] = Wih_lo.T
        # B half: rhs rows [x; h], out cols 64:128
        Wd[0:64, 1, s, 0, 64:128] = Wih_hi.T
        Wd[64:128, 1, s, 0, 64:128] = Whh_hi.T
        Wd[0:64, 1, s, 1, 64:128] = Wih_hi.T
        Wd[64, 1, s, 1, 64:128] = b_hi
        Wd[65, 1, s, 1, 64:128] = b_lo
        Wd[0:64, 1, s, 2, 64:128] = Wih_lo.T
        Wd[64:128, 1, s, 2, 64:128] = Whh_lo.T
    WO = np.zeros((128, 2), np.float32)
    WO[0:64, 0] = W_out[0].astype(np.float32)
    WO[64:128, 1] = W_out[0].astype(np.float32)
    BO = np.full((2, 1), np.float32(b_out[0]))
    return {
        "Wd": Wd.astype(e4),
        "WOd": WO.astype(bf16),
        "BOd": BO,
    }


_BUILD_CACHE = {}


def build_nc(merged=True):
    key = ("nc", merged)
    if key in _BUILD_CACHE:
        return _BUILD_CACHE[key]
    nc = bacc.Bacc("TRN2", target_bir_lowering=False, debug=False)
    aps = {
        "xt": nc.dram_tensor("xt", [T, INP, 2, B_LOC], F8,
                             kind="ExternalInput").ap(),
        "xinit": nc.dram_tensor("xinit", [2, 128, 2, 2, B_LOC // 2], F8,
                                kind="ExternalInput").ap(),
        "Wd": nc.dram_tensor("Wd", [128, 2, 4, 3, 128], F8,
                             kind="ExternalInput").ap(),
        "WOd": nc.dram_tensor("WOd", [128, 2], BF, kind="ExternalInput").ap(),
        "BOd": nc.dram_tensor("BOd", [2, 1], F32, kind="ExternalInput").ap(),
        "y": nc.dram_tensor("y", [B_LOC], F32, kind="ExternalOutput").ap(),
    }
    with tile.TileContext(nc) as tc:
        emit_lstm(tc, aps)
    nc.compile()
    _BUILD_CACHE[key] = nc
    return nc


def make_in_maps(x, W_ih, W_hh, b_ih, b_hh, W_out, b_out, merged=True):
    e4 = ml_dtypes.float8_e4m3
    wd = prep_weights(W_ih, W_hh, b_ih, b_hh, W_out, b_out)
    # [B, T, I] -> [T, I, B]; hi/lo e4m3 split stacked on a new axis
    xT = np.ascontiguousarray(x.transpose(1, 2, 0)).astype(np.float32)
    hi = xT.astype(e4).astype(np.float32)
    lo = (xT - hi).astype(e4)
    xt = np.stack([hi.astype(e4), lo], axis=2)  # [T, I, 2, B]

    # whole-tile init images [par, k, kind, half, B/2]: x(t=par) + h=0 +
    # lo-slot constants (ones rows for the bias, zeros elsewhere)
    B_half = B // 2
    xinit = np.zeros((2, 128, 2, 2, B_half), e4)
    xg = xt.reshape(T, INP, 2, B // (2 * NB), 2, NB)  # [...groups, half, col]
    for p in range(2):
        xA = xg[p, :, :, :, 0, :].reshape(INP, 2, B_half)
        xB = xg[p, :, :, :, 1, :].reshape(INP, 2, B_half)
        xinit[p, 64:128, :, 0, :] = xA  # A half: x at rows 64:128, both kinds
        xinit[p, 0:64, :, 1, :] = xB    # B half: x at rows 0:64
        xinit[p, 0, 1, 0, :] = e4(1.0)   # A lo ones-rows (bias)
        xinit[p, 1, 1, 0, :] = e4(1.0)
        xinit[p, 64, 1, 1, :] = e4(1.0)  # B lo ones-rows
        xinit[p, 65, 1, 1, :] = e4(1.0)

    in_maps = []
    for c in range(NCORES):
        sl = np.ascontiguousarray(xt[:, :, :, c * B_LOC:(c + 1) * B_LOC])
        sli = np.ascontiguousarray(
            xinit[:, :, :, :, c * (B_LOC // 2):(c + 1) * (B_LOC // 2)])
        in_maps.append({"xt": sl, "xinit": sli, **wd})
    return in_maps


def kernel(x, W_ih, W_hh, b_ih, b_hh, W_out, b_out):
    from concourse.bass_utils import run_bass_kernel_spmd

    nc = build_nc()
    in_maps = make_in_maps(x, W_ih, W_hh, b_ih, b_hh, W_out, b_out)
    res = run_bass_kernel_spmd(nc, in_maps, core_ids=list(range(NCORES)))
    y = np.concatenate([res.results[c]["y"] for c in range(NCORES)])
    return y.reshape(B, 1).astype(np.float32)
